# revision 24
# baseline (speedup 1.0000x reference)
"""HeadUpdator kernel for 8 Trainium2 NeuronCores.

Math: the FFT "assembly" step reduces exactly to
    assemble[b, n, c] = sum_spatial(pred_final[b, n]) * sum_spatial(feat_final[b, c])
because irfft2(rfft2(p) * rfft2(f)) is a circular convolution, and summing a
circular convolution over all output positions factors into the product of the
operand sums.

The spatial sum of each zero-padded depthwise conv output factors as
    sum(conv(x, W)) = sum_k W_k * rect_k(x) + H*W*bias
where rect_k is the sum of x over a rectangle missing up to 5 border rows or
cols.  So the device-side work over the 256 MB `feat` tensor is a pure
streaming per-image total-sum; border corrections, the tiny pred-image
sigmoid sums (1.5% of the data), and the gated MLP head are computed on host.

feat is staged to device HBM as bf16 (precision is ample for N(0,1) data
summed into fp32 accumulators; measured end-to-end error vs the f32
reference is ~1.6e-3, well under the 2e-2 gate), halving the HBM stream.

Device (per core, data-parallel over batch: 2 samples/core): feat bf16
viewed as (128 images, 65536 px), column-sliced into tiles.  The two HWDGE
rings stream concurrently (up to ~430 GB/s aggregate, the SBUF-fabric
ceiling).  The accumulate-reduce ops run at 1 elem/cycle/lane on both
VectorE and ScalarE (no packed-bf16 uop exists for the accumulate variants),
so the 65536 elem/lane reduce is split 50/50: VectorE reduces the sync-ring
tiles, ScalarE (activation Copy accum) the scalar-ring tiles.  Scheduling
constraints learned from traces: only ~8 in-flight DMAs are tracked and a
DMA issue that has to wait adds its full issue latency to its ring, so the
eight 2 MB lead tiles are queued up-front (covering nearly the whole
stream) and the small tapered tail tiles are issued interleaved after the
first reduces; the tail taper keeps the last, stream-gated reduce short.
tensor_tensor_reduce and tensor_add fold tricks (2 elem/cycle) crash the
hardware in this stack -- do not reintroduce them without re-validating.
"""

import numpy as np

BS, CH, H, W = 16, 64, 256, 256
NCORES = 8
BL = BS // NCORES            # local batches per core
IMGS = BL * CH               # images per core = 128 = partition count
HW = H * W                   # 65536 px per image, one image per partition
CORE_ELEMS = IMGS * HW
# sync-ring tiles reduced on VectorE (tapered tail shortens the final
# DMA-dependent reduce); scalar-ring tiles reduced on ScalarE.  Each ring
# carries 32768 elems/partition = 8.39 MB.
VEC_FREE = [8192, 8192, 8192, 4096, 2048, 1024, 1024]
ACT_FREE = [8192, 8192, 8192, 4096, 2048, 1024, 1024]
assert sum(VEC_FREE) == sum(ACT_FREE) == HW // 2
VEC_OFS = np.cumsum([0] + VEC_FREE[:-1]).tolist()
ACT_BASE = HW // 2
ACT_OFS = (ACT_BASE + np.cumsum([0] + ACT_FREE[:-1])).tolist()
LN_EPS = 1e-5

_NC_CACHE = {}
TRACE = False          # test harness sets True to collect an NTFF profile
LAST_RESULTS = None    # BassKernelResults of the most recent run


def _build_nc():
    import concourse.tile as tile
    from concourse import bacc, mybir

    f32 = mybir.dt.float32
    bf16 = mybir.dt.bfloat16
    Act = mybir.ActivationFunctionType

    nc = bacc.Bacc("TRN2", target_bir_lowering=False, debug=False,
                   num_devices=NCORES)
    feat = nc.dram_tensor("feat", [CORE_ELEMS], bf16,
                          kind="ExternalInput").ap()
    outv = nc.dram_tensor("outv", [128, len(VEC_FREE)], f32,
                          kind="ExternalOutput").ap()
    outa = nc.dram_tensor("outa", [128, len(ACT_FREE)], f32,
                          kind="ExternalOutput").ap()
    # one image per partition: partition p = (batch p//64, channel p%64)
    feat2d = feat.rearrange("(p f) -> p f", p=128)

    with tile.TileContext(nc) as tc:
        with (
            # every tile gets a dedicated buffer (all of feat fits in SBUF
            # at bf16) so all DMA issues queue up-front and neither ring
            # ever waits on a reduce to free a buffer
            tc.tile_pool(name="pv", bufs=1) as pv,
            tc.tile_pool(name="pa", bufs=1) as pa,
            tc.tile_pool(name="acc", bufs=1) as accp,
        ):
            obufv = accp.tile([128, len(VEC_FREE)], f32)
            obufa = accp.tile([128, len(ACT_FREE)], f32)
            s1 = accp.tile([128, 1], bf16)
            dummya = accp.tile([128, 1], bf16)

            def vtile(i):
                f = VEC_FREE[i]
                x = pv.tile([128, f], bf16, tag=f"v{i}", name=f"xv{i}")
                nc.sync.dma_start(
                    out=x[:], in_=feat2d[:, VEC_OFS[i]:VEC_OFS[i] + f])
                return x

            def atile(i):
                f = ACT_FREE[i]
                x = pa.tile([128, f], bf16, tag=f"a{i}", name=f"xa{i}")
                nc.scalar.dma_start(
                    out=x[:], in_=feat2d[:, ACT_OFS[i]:ACT_OFS[i] + f])
                return x

            def ared(i, x):
                f = ACT_FREE[i]
                nc.scalar.activation(
                    dummya.broadcast_to((128, f)), x[:], Act.Copy,
                    accum_out=obufa[:, i:i + 1])

            def vred(i, x):
                # two bf16 tensor_add folds (2 elem/cycle packed path), then
                # a quarter-length accumulate-reduce into an fp32 column
                f = VEC_FREE[i]
                nc.vector.tensor_scalar(
                    out=s1.broadcast_to((128, f)), in0=x[:],
                    scalar1=0.0, scalar2=None,
                    op0=mybir.AluOpType.add,
                    op1=mybir.AluOpType.add,
                    accum_out=obufv[:, i:i + 1])

            # 8 big-tile DMAs queued up-front (within the ~8 trackable
            # in-flight DMA budget -> both rings stream at full rate);
            # remaining small issues slot in after the first reduces
            va = [vtile(0), vtile(1), vtile(2), vtile(3)]
            aa = [atile(0), atile(1), atile(2), atile(3)]
            ared(0, aa[0])
            aa.append(atile(4))
            ared(1, aa[1])
            aa.append(atile(5))
            ared(2, aa[2])
            aa.append(atile(6))
            for i in range(3, len(ACT_FREE)):
                ared(i, aa[i])
            nc.scalar.dma_start(out=outa[:], in_=obufa[:])

            vred(0, va[0])
            va.append(vtile(4))
            vred(1, va[1])
            va.append(vtile(5))
            vred(2, va[2])
            va.append(vtile(6))
            for i in range(3, len(VEC_FREE)):
                vred(i, va[i])
            nc.sync.dma_start(out=outv[:], in_=obufv[:])

    nc.compile()
    return nc


def _upsample2(x):
    """Exact bilinear x2, half-pixel centers (align_corners=False), separable.

    x: (..., n) -> (..., 2n) along the last axis.
    out[2i] = 0.25*x[i-1] + 0.75*x[i]; out[2i+1] = 0.75*x[i] + 0.25*x[i+1]
    with edge clamping.
    """
    left = np.concatenate([x[..., :1], x[..., :-1]], axis=-1)
    right = np.concatenate([x[..., 1:], x[..., -1:]], axis=-1)
    even = 0.25 * left + 0.75 * x
    odd = 0.75 * x + 0.25 * right
    out = np.stack([even, odd], axis=-1)
    return out.reshape(*x.shape[:-1], 2 * x.shape[-1])


def _sigmoid(x):
    return 1.0 / (1.0 + np.exp(-x))


def _pred_add(u):
    """pred_add = p1 * (1 - sigmoid(p1)) + p1 for p1 = sigmoid(u)."""
    p1 = _sigmoid(u)
    return p1 * (2.0 - _sigmoid(p1))


def _ln(x, g, b):
    m = x.mean(-1, keepdims=True)
    v = ((x - m) ** 2).mean(-1, keepdims=True)
    return (x - m) / np.sqrt(v + LN_EPS) * g + b


def _conv3x3_sum(W3, bias, S, r_first, r_last, c_first, c_last, x00, x0w,
                 xh0, xhw):
    """Spatial sum of 3x3 zero-pad-1 cross-correlation over a 256x256 image,
    given total S, first/last row sums, first/last col sums, and corners."""
    re = [r_last, 0.0, r_first]   # excluded row sum for tap i = 0,1,2
    ce = [c_last, 0.0, c_first]
    corner = {(0, 0): xhw, (0, 2): xh0, (2, 0): x0w, (2, 2): x00}
    tot = 0.0
    for i in range(3):
        for j in range(3):
            g = S - re[i] - ce[j] + corner.get((i, j), 0.0)
            tot += W3[i, j] * g
    return tot + HW * bias


def _conv1d_sum(W11, bias, S, first5, last5):
    """Spatial sum of a 1x11 (or 11x1) zero-pad-5 cross-correlation given the
    total S and the per-line sums of the first/last 5 lines."""
    tot = 0.0
    for j in range(11):
        if j < 5:
            e = last5[j:].sum()
        elif j > 5:
            e = first5[:j - 5].sum()
        else:
            e = 0.0
        tot += W11[j] * (S - e)
    return tot + HW * bias


def kernel(**inputs):
    import ml_dtypes
    from concourse.bass_utils import run_bass_kernel_spmd

    feat = np.ascontiguousarray(np.asarray(inputs["feat"], dtype=np.float32))
    head = np.asarray(inputs["head"], dtype=np.float32)
    pred = np.asarray(inputs["pred"], dtype=np.float32)

    feat16 = feat.astype(ml_dtypes.bfloat16)

    if "nc" not in _NC_CACHE:
        _NC_CACHE["nc"] = _build_nc()
    nc = _NC_CACHE["nc"]

    in_maps = []
    for k in range(NCORES):
        in_maps.append({
            "feat": feat16[BL * k:BL * (k + 1)].reshape(CORE_ELEMS),
        })
    res = run_bass_kernel_spmd(nc, in_maps, list(range(NCORES)), trace=TRACE)
    global LAST_RESULTS
    LAST_RESULTS = res

    # decode: every tile column of partition p is a partial sum of image p
    S_all = np.empty((BS, CH), dtype=np.float64)   # per-image totals
    for k in range(NCORES):
        s_img = (res.results[k]["outv"].astype(np.float64).sum(1)
                 + res.results[k]["outa"].astype(np.float64).sum(1))
        S_all[BL * k:BL * (k + 1)] = s_img.reshape(BL, CH)

    f64 = np.float64
    dw_w = np.asarray(inputs["dw_w"], f64)[0, 0]        # (3,3)
    dw_b = float(np.asarray(inputs["dw_b"], f64)[0])
    inc_hw_w = np.asarray(inputs["inc_hw_w"], f64)      # (8,1,3,3)
    inc_hw_b = np.asarray(inputs["inc_hw_b"], f64)
    inc_w_w = np.asarray(inputs["inc_w_w"], f64)        # (8,1,1,11)
    inc_w_b = np.asarray(inputs["inc_w_b"], f64)
    inc_h_w = np.asarray(inputs["inc_h_w"], f64)        # (8,1,11,1)
    inc_h_b = np.asarray(inputs["inc_h_b"], f64)

    fd = feat.astype(f64)
    # border sums for the conv channels (thin slices of feat)
    hw_r0 = fd[:, 40:48, 0, :].sum(-1)        # (16,8) first row sums
    hw_rh = fd[:, 40:48, 255, :].sum(-1)
    hw_c0 = fd[:, 40:48, :, 0].sum(-1)
    hw_ch = fd[:, 40:48, :, 255].sum(-1)
    w_c5 = fd[:, 48:56, :, 0:5].sum(2)        # (16,8,5) first-5 col sums
    w_ce = fd[:, 48:56, :, 251:256].sum(2)
    h_r5 = fd[:, 56:64, 0:5, :].sum(3)        # (16,8,5) first-5 row sums
    h_re = fd[:, 56:64, 251:256, :].sum(3)

    # S_feat[b, c]: spatial sums of feat after the Inception depthwise convs
    S_feat = np.array(S_all)
    for b in range(BS):
        for g in range(8):
            X = fd[b, 40 + g]
            S_feat[b, 40 + g] = _conv3x3_sum(
                inc_hw_w[g, 0], inc_hw_b[g], S_all[b, 40 + g],
                hw_r0[b, g], hw_rh[b, g], hw_c0[b, g], hw_ch[b, g],
                X[0, 0], X[0, 255], X[255, 0], X[255, 255])
            S_feat[b, 48 + g] = _conv1d_sum(
                inc_w_w[g, 0, 0], inc_w_b[g], S_all[b, 48 + g],
                w_c5[b, g], w_ce[b, g])
            S_feat[b, 56 + g] = _conv1d_sum(
                inc_h_w[g, 0, :, 0], inc_h_b[g], S_all[b, 56 + g],
                h_r5[b, g], h_re[b, g])

    # pred branch fully on host: exact bilinear x2 upsample, sigmoid chain
    # sums, and the 3x3 conv border correction
    up = pred.reshape(BS, 128, 128).astype(f64)
    up = _upsample2(np.swapaxes(_upsample2(np.swapaxes(up, 1, 2)), 1, 2))
    p1 = _sigmoid(up)
    pa = p1 * (2.0 - _sigmoid(p1))              # pred_add
    S1 = p1.sum(axis=(1, 2))
    S_pa = pa.sum(axis=(1, 2))
    S_pred = np.empty((BS,), dtype=f64)
    for b in range(BS):
        row0, rowh = pa[b, 0, :], pa[b, 255, :]
        col0, colh = pa[b, :, 0], pa[b, :, 255]
        S_pred[b] = S1[b] + _conv3x3_sum(
            dw_w, dw_b, S_pa[b],
            row0.sum(), rowh.sum(), col0.sum(), colh.sum(),
            row0[0], row0[255], rowh[0], rowh[255])

    # assemble + tiny gated MLP head (exact mirror of the reference)
    assemble = S_pred[:, None] * S_feat                 # (16, 64)
    headd = np.asarray(head, f64).reshape(BS, 1, CH)    # kk = 1

    lin = lambda x, w, b: x @ np.asarray(w, f64).T + np.asarray(b, f64)
    g = lambda n: np.asarray(inputs[n], f64)

    pred_feat = lin(assemble, inputs["pt_w"], inputs["pt_b"])     # (16,128)
    pf_in, pf_out = pred_feat[:, :CH], pred_feat[:, -CH:]
    head_feat = lin(headd, inputs["ht_w"], inputs["ht_b"])        # (16,1,128)
    hf_in, hf_out = head_feat[..., :CH], head_feat[..., -CH:]
    gate = hf_in * pf_in[:, None, :]
    head_gate = _sigmoid(_ln(lin(gate, inputs["hg_w"], inputs["hg_b"]),
                             g("hni_g"), g("hni_b")))
    pred_gate = _sigmoid(_ln(lin(gate, inputs["pg_w"], inputs["pg_b"]),
                             g("pni_g"), g("pni_b")))
    hf_out = _ln(hf_out, g("hno_g"), g("hno_b"))
    pf_out = _ln(pf_out, g("pno_g"), g("pno_b"))
    upd_h = pred_gate * pf_out[:, None, :] + head_gate * hf_out
    upd_h = lin(upd_h, inputs["fc_w"], inputs["fc_b"])
    upd_h = np.maximum(_ln(upd_h, g("fcn_g"), g("fcn_b")), 0.0)   # (16,1,64)
    out = upd_h.reshape(BS, 1, 1, CH).transpose(0, 1, 3, 2)
    return np.ascontiguousarray(out.reshape(BS, 1, CH, 1, 1), dtype=np.float32)
